# revision 26
# baseline (speedup 1.0000x reference)
"""Single-head causal attention prefill with inline RoPE on 8 trn2 NeuronCores.

Full inputs:  x [8, 2048, 1024], Wq/Wk/Wv [64, 1024]  (all fp32)
Full outputs: (out, k, v) each [8, 2048, 64] fp32  (k is post-RoPE, v raw)

Sharding: data-parallel over batch B=8 -> one batch element per core.

v2 redesign vs the staged baseline (sim device time 204us -> 58.7us):
  * all matmul operands bf16 (1 cyc/row on PE vs fp32's 4); PSUM stays fp32
  * x / trig tables / weights shipped bf16 (halves input DMA); outputs bf16,
    upcast to fp32 on host (rel err ~8e-3 vs the 2e-2 gate)
  * 4-stage software pipeline over 512-wide q tiles: stage n runs attention
    (tile n vs kv blocks 0..4n+3) while stage n+1's projection matmuls fill
    PE gaps and stage n+1's rope chain resolves mid-stage, not at the
    boundary; scores+exp for unit u+1 are emitted before the PVs of unit u
    so ACT always has input ready
  * off-diagonal score chunks paired into [128,1024] PSUM tiles -> one exp
    per pair (halves ACT per-instruction overhead)
  * 14 dummy warmup matmuls burn the PE p-state ramp (~7us to full clock)
    while the input DMAs stream in, so real work starts at full speed
  * all input DMAs issue in priority order from the single ACT queue (HWDGE
    costs ~630ns/DMA serialized and transfers are exclusive); weight images
    are precomputed host-side in SBUF layout so each is one >=512B-elem DMA;
    k/v/out outputs are staged in SBUF and written as one batched DMA per
    stage; GPSIMD (Pool) never touches PSUM (hardware rejects it)
  * ACT runs exp only (plus a preload to hide the 1283ns table load); the
    exp->PV chain is decoupled via bf16 pt tiles; softmax rowsums ride as a
    ones-row in the [V|1] PV stationary
  * head dims stay in natural interleaved order; RoPE pair-swap is a PE
    permute-matmul with perm[h^1, h] = 1 and host trig tables t1/t2 (shipped
    as 64 rows, broadcast to the k half on DVE)
"""

import numpy as np

import concourse.bass as bass
import concourse.mybir as mybir
import concourse.tile as tile
from concourse.vector_clock import ScopedClock, VectorClock

B = 8
T = 2048
C = 1024
HS = 64
NCORES = 8
FP32 = mybir.dt.float32
BF16 = mybir.dt.bfloat16
NT = T // 512  # 4 q tiles of 512
NJ = T // 128  # 16 kv blocks of 128
NC_CHUNKS = C // 128  # 8 contraction chunks
EMIT_MARKS = []  # (instruction_count_so_far, label) for trace attribution


def _mark(nc, label):
    f = nc.m.functions[0]
    EMIT_MARKS.append((sum(len(b.instructions) for b in f.blocks), label))


class SplitDrainTileContext(tile.TileContext):
    """Walrus in this environment rejects >1 semaphore wait per instruction,
    but Tile's kernel-tail drain wants one wait per live proc. Absorb the
    global clock into the SP engine through a chain of nops first, so the
    drain itself needs no waits."""

    def _drain_and_barrier(self, tick_clock, wait_clock):
        vc = tick_clock.global_clock
        n = len(vc)
        absorbed = VectorClock([0] * n)
        for i in range(n):
            if vc[i] <= 0:
                continue
            target = absorbed.copy()
            target.require_at_least(i, vc[i])
            nop = self.nc.sync.nop()
            wait_clock.add_sem_waits(
                nop.ins,
                ScopedClock({None: target.copy()}),
                ScopedClock({None: absorbed.copy()}),
            )
            absorbed = target
        drain_inst = self.nc.sync.drain()
        wait_clock.add_sem_waits(
            drain_inst.ins,
            ScopedClock({None: tick_clock.global_clock.copy()}),
            ScopedClock({None: absorbed.copy()}),
        )
        self.nc.all_engine_barrier()
        assert self.sems is not None
        popped = self.nc._tile_sem_poison_stack.pop()
        assert popped is self._sem_poison
        self.nc.clear_and_free_semaphores(list(self.sems.allocated().values()))
        self.nc.all_engine_barrier()


def _emit(tc, ctx):
    nc = tc.nc
    xT = nc.dram_tensor("xT", [C, T], BF16, kind="ExternalInput").ap()
    # weight images already in SBUF layout (partition-major, chunk-major cols)
    wqkd = nc.dram_tensor("wqkd", [128, C], BF16, kind="ExternalInput").ap()
    wvd = nc.dram_tensor("wvd", [128, NC_CHUNKS * HS], BF16, kind="ExternalInput").ap()
    t1d = nc.dram_tensor("t1", [64, T], BF16, kind="ExternalInput").ap()
    t2d = nc.dram_tensor("t2", [64, T], BF16, kind="ExternalInput").ap()
    permTd = nc.dram_tensor("permT", [128, 128], BF16, kind="ExternalInput").ap()
    dmaskd = nc.dram_tensor("dmask", [128, 128], BF16, kind="ExternalInput").ap()
    identd = nc.dram_tensor("identd", [128, 128], BF16, kind="ExternalInput").ap()
    out_d = nc.dram_tensor("out", [T, HS], BF16, kind="ExternalOutput").ap()
    k_d = nc.dram_tensor("k", [T, HS], BF16, kind="ExternalOutput").ap()
    v_d = nc.dram_tensor("v", [T, HS], BF16, kind="ExternalOutput").ap()

    consts = ctx.enter_context(tc.tile_pool(name="consts", bufs=1))
    xall = consts.tile([128, NC_CHUNKS * T], BF16, tag="xall")  # block n: cols n*4096+512c
    wqk_s = consts.tile([128, C], BF16, tag="wqk")  # chunk c at [:, 128c:128c+128]
    wv_s = consts.tile([128, NC_CHUNKS * HS], BF16, tag="wv")
    t1_s = consts.tile([128, T], BF16, tag="t1")
    t2_s = consts.tile([128, T], BF16, tag="t2")
    perm_s = consts.tile([128, 128], BF16, tag="perm")
    dmask_s = consts.tile([128, 128], BF16, tag="dmask")
    ident = consts.tile([128, 128], BF16, tag="ident")
    q_roped = consts.tile([64, T], BF16, tag="qroped")
    kT_s = consts.tile([64, T], BF16, tag="kT")
    vT_s = consts.tile([64, T], BF16, tag="vT")
    vones_s = consts.tile([128, NJ * (HS + 1)], BF16, tag="vones")

    # All input DMAs issue from the single ACT queue: HWDGE round-robins
    # between engine queues and DMA transfers serialize, so one queue in
    # priority order (weights, x block 0, trig tables, remaining x) is the
    # only way to control arrival order.
    def dma_x_block(n):
        nc.scalar.dma_start(
            xall[:, 4096 * n : 4096 * (n + 1)].rearrange(
                "p (c f) -> p c f", c=NC_CHUNKS
            ),
            xT[:, 512 * n : 512 * (n + 1)].rearrange("(c p) f -> p c f", p=128),
        )

    # x block 0 in two halves so the first projection matmuls start sooner;
    # the warmup matmuls keep the PE p-state hot across any arrival gaps.
    # Order: x0a, wqk, x0b, wv — each transfer arrives just before the
    # matmuls that need it.
    nc.scalar.dma_start(
        xall[:, 0:2048].rearrange("p (c f) -> p c f", c=4),
        xT[0:512, 0:512].rearrange("(c p) f -> p c f", p=128),
    )
    nc.scalar.dma_start(wqk_s[:, :], wqkd)
    nc.scalar.dma_start(
        xall[:, 2048:4096].rearrange("p (c f) -> p c f", c=4),
        xT[512:1024, 0:512].rearrange("(c p) f -> p c f", p=128),
    )
    nc.scalar.dma_start(wv_s[:, :], wvd)
    nc.scalar.dma_start(t1_s[0:64, :], t1d)
    nc.scalar.dma_start(t2_s[0:64, :], t2d)
    # q and k halves share the trig rows: broadcast on idle DVE
    nc.vector.tensor_copy(t1_s[64:128, :], t1_s[0:64, :])
    nc.vector.tensor_copy(t2_s[64:128, :], t2_s[0:64, :])
    nc.scalar.dma_start(perm_s[:, :], permTd)
    nc.scalar.dma_start(dmask_s[:, :], dmaskd)
    nc.scalar.dma_start(ident[:, :], identd)
    for n in range(1, NT):
        dma_x_block(n)
    nc.gpsimd.memset(vones_s[:, :], 1.0)
    # Preload the Exp activation table while ACT is otherwise idle so the
    # 1283ns table load is off the first real softmax's critical path.
    scratch = consts.tile([1, 1], FP32, tag="scratch")
    nc.gpsimd.memset(scratch[:, :], 0.0)
    nc.scalar.activation(
        scratch[:, :], scratch[:, :], mybir.ActivationFunctionType.Exp
    )
    # PE p-state warm-up: the tensor engine needs ~7us of uninterrupted work
    # before it reaches full clock. Burn that transient on dummy matmuls while
    # the x DMAs stream in, so the real projection starts at full speed.
    wu_sb = consts.tile([128, 640], BF16, tag="wu")
    nc.gpsimd.memset(wu_sb[:, :], 0.0)

    proj_psum = ctx.enter_context(tc.tile_pool(name="proj_psum", bufs=1, space="PSUM"))
    v_psum = ctx.enter_context(tc.tile_pool(name="v_psum", bufs=1, space="PSUM"))
    o_psum = ctx.enter_context(tc.tile_pool(name="o_psum", bufs=2, space="PSUM"))
    st_psum = ctx.enter_context(tc.tile_pool(name="st_psum", bufs=2, space="PSUM"))
    qks_pool = ctx.enter_context(tc.tile_pool(name="qks", bufs=8))
    pt_pool = ctx.enter_context(tc.tile_pool(name="pt", bufs=8))
    kn_pool = ctx.enter_context(tc.tile_pool(name="kn", bufs=4))
    osb_pool = ctx.enter_context(tc.tile_pool(name="osb", bufs=3))
    outs_pool = ctx.enter_context(tc.tile_pool(name="outs", bufs=4))
    rc_pool = ctx.enter_context(tc.tile_pool(name="rc", bufs=3))

    def emit_proj_thunks(n):
        """Projection matmuls for q tile n as single-matmul thunks so they can
        be interleaved as PE filler into the previous stage's attention."""
        qk_ps = proj_psum.tile([128, 512], FP32, tag="proj", name=f"qk_ps{n}")
        v_ps = v_psum.tile([64, 512], FP32, tag="v", name=f"v_ps{n}")
        xsl = lambda c: xall[:, 4096 * n + 512 * c : 4096 * n + 512 * (c + 1)]
        thunks = []
        for c in range(NC_CHUNKS):
            thunks.append(
                lambda c=c: nc.tensor.matmul(
                    qk_ps[:, :], wqk_s[:, 128 * c : 128 * (c + 1)], xsl(c),
                    start=(c == 0), stop=(c == NC_CHUNKS - 1),
                )
            )
        for c in range(NC_CHUNKS):
            thunks.append(
                lambda c=c: nc.tensor.matmul(
                    v_ps[:, :], wv_s[:, HS * c : HS * (c + 1)], xsl(c),
                    start=(c == 0), stop=(c == NC_CHUNKS - 1),
                )
            )
        return (qk_ps, v_ps), thunks

    def emit_rope(n, qk_ps, v_ps, qkw_ps, kvbuf):
        """Rope for tile n. vT copy + v transposes first (independent of the
        qk permute chain) so PE has work while the Pool->PE->DVE rope latency
        chain drains; m2 reads the permuted PSUM directly."""
        sl = slice(512 * n, 512 * (n + 1))
        qk_sb = qks_pool.tile([128, 512], BF16, tag="qksb", name=f"qk_sb{n}")
        nc.vector.tensor_copy(qk_sb[:, :], qk_ps[:, :])
        nc.vector.tensor_copy(vT_s[:, sl], v_ps[:, :])
        nc.tensor.matmul(qkw_ps[:, :], perm_s[:, :], qk_sb[:, :], start=True, stop=True)
        for j in range(4 * n, 4 * n + 4):
            emit_v_out(j, kvbuf)
        m1 = qks_pool.tile([128, 512], BF16, tag="qksb", name=f"m1_{n}")
        m2 = qks_pool.tile([128, 512], BF16, tag="qksb", name=f"m2_{n}")
        nc.vector.tensor_mul(m1[:, :], qk_sb[:, :], t1_s[:, sl])
        nc.vector.tensor_mul(m2[:, :], qkw_ps[:, :], t2_s[:, sl])
        nc.vector.tensor_add(q_roped[:, sl], m1[0:64, :], m2[0:64, :])
        nc.vector.tensor_add(kT_s[:, sl], m1[64:128, :], m2[64:128, :])

    def emit_v_out(j, kvbuf):
        """Transpose v block j to natural layout; stage rows; fill vones."""
        bsl = slice(128 * j, 128 * (j + 1))
        u = j % 4
        vtr = st_psum.tile([128, HS], BF16, tag="st", name=f"vtr{j}")
        nc.tensor.transpose(vtr[:, :], vT_s[:, bsl], ident[0:64, 0:64])
        vsl = slice((HS + 1) * j, (HS + 1) * j + HS)
        nc.vector.tensor_copy(vones_s[:, vsl], vtr[:, :])
        nc.gpsimd.tensor_copy(kvbuf[:, HS * (4 + u) : HS * (5 + u)], vones_s[:, vsl])

    def emit_k_out(j, kvbuf):
        """Transpose roped-k block j to natural layout; stage rows."""
        bsl = slice(128 * j, 128 * (j + 1))
        u = j % 4
        ktr = st_psum.tile([128, HS], BF16, tag="st", name=f"ktr{j}")
        nc.tensor.transpose(ktr[:, :], kT_s[:, bsl], ident[0:64, 0:64])
        nc.vector.tensor_copy(kvbuf[:, HS * u : HS * (u + 1)], ktr[:, :])

    def emit_scores_exp(n, j0, paired):
        """Scores + one exp for q tile n vs kv block j0 (and j0+1 if paired,
        sharing one [128,1024] st tile). Returns (pt, offs) for the PVs."""
        js = [j0, j0 + 1] if paired else [j0]
        st = st_psum.tile([128, 1024], FP32, tag="st")
        pt = pt_pool.tile([128, 1024], BF16, tag="pt")
        offs = []
        with tc.high_priority(offset=400):
            for idx, j in enumerate(js):
                base = 512 * idx
                s0 = 128 * (j % 4) if j // 4 == n else 0
                nc.tensor.matmul(
                    st[:, base + s0 : base + 512], kT_s[:, 128 * j : 128 * (j + 1)],
                    q_roped[:, 512 * n + s0 : 512 * (n + 1)], start=True, stop=True,
                )
                offs.append((j, base, s0))
            if paired:  # off-diagonal pair: both halves fully written
                nc.scalar.activation(
                    pt[:, 0:1024], st[:, 0:1024], mybir.ActivationFunctionType.Exp
                )
            else:
                j, base, s0 = offs[0]
                nc.scalar.activation(
                    pt[:, s0:512], st[:, s0:512], mybir.ActivationFunctionType.Exp
                )
        for j, base, s0 in offs:
            if j // 4 == n:
                nc.vector.tensor_mul(
                    pt[:, base + s0 : base + s0 + 128],
                    pt[:, base + s0 : base + s0 + 128],
                    dmask_s[:, :],
                )
        return pt, offs

    def emit_pv(n, o_ps, pt, offs):
        for j, base, s0 in offs:
            nc.tensor.matmul(
                o_ps[:, s0:512], vones_s[:, (HS + 1) * j : (HS + 1) * (j + 1)],
                pt[:, base + s0 : base + 512], start=(j == 0), stop=(j == 4 * n + 3),
            )

    def emit_finalize(n, o_ps):
        """Transpose o tile back to [t, h], normalize by rowsum, DMA out."""
        osb = osb_pool.tile([HS + 1, 512], BF16, tag="osb")
        nc.vector.tensor_copy(osb[:, :], o_ps[:, :])
        obuf = outs_pool.tile([128, 4 * HS], BF16, tag="ou", name=f"obuf{n}")
        for u in range(4):
            ot = st_psum.tile([128, HS + 1], BF16, tag="st", name=f"ot{n}_{u}")
            nc.tensor.transpose(
                ot[:, :], osb[:, 128 * u : 128 * (u + 1)], ident[0 : HS + 1, 0 : HS + 1]
            )
            rc = rc_pool.tile([128, 1], FP32, tag="rc")
            nc.vector.reciprocal(rc[:, :], ot[:, HS : HS + 1])
            nc.vector.tensor_scalar_mul(
                obuf[:, HS * u : HS * (u + 1)], ot[:, 0:HS], rc[:, :]
            )
        nc.sync.dma_start(
            out_d[512 * n : 512 * (n + 1), :].rearrange("(j p) h -> p j h", p=128),
            obuf[:, :].rearrange("p (j h) -> p j h", j=4),
        )

    # ---- software pipeline over stages n = 0..3 ----
    # stage n: rope/transpose for tile n, then attention for tile n vs kv
    # blocks 0..4n+3, with stage n+1's projection matmuls interleaved as PE
    # filler wherever this stage's PE stream would otherwise stall.
    for t in range(14):
        wu = st_psum.tile([128, 512], FP32, tag="st", name=f"wu{t}")
        nc.tensor.matmul(wu[:, :], wu_sb[:, 0:128], wu_sb[:, 128:640], start=True, stop=True)

    (qk_ps, v_ps), thunks = emit_proj_thunks(0)
    for th in thunks:
        th()
    qkw_ps = proj_psum.tile([128, 512], FP32, tag="proj", name="qkw_ps0")
    kvbuf = kn_pool.tile([128, 8 * HS], BF16, tag="kn", name="kvbuf0")
    emit_rope(0, qk_ps, v_ps, qkw_ps, kvbuf)
    pending_final = None

    for n in range(NT):
        if n + 1 < NT:
            nxt_ps, fillers = emit_proj_thunks(n + 1)
        else:
            nxt_ps, fillers = None, []
        fstate = {"i": 0}

        def fill(cnt, fillers=fillers, fstate=fstate):
            for _ in range(cnt):
                if fstate["i"] < len(fillers):
                    fillers[fstate["i"]]()
                    fstate["i"] += 1

        units = [(p0, True) for p0 in range(0, 4 * n, 2)]
        units += [(j, False) for j in range(4 * n, 4 * n + 4)]
        o_ps = o_psum.tile([HS + 1, 512], FP32, tag="o", name=f"o_ps{n}")
        ui_rope = max(2, (2 * len(units)) // 3)  # where next stage's rope goes
        # software-pipelined: scores+exp for unit u+1 are emitted BEFORE the
        # PVs of unit u, so ACT always has its next input ready and the
        # insertions (finalize / k-out / filler / next rope) never starve it.
        pending_pv = emit_scores_exp(n, *units[0])
        for ui in range(len(units)):
            _mark(nc, f"s{n}.attn")
            if ui + 1 < len(units):
                nxt_pv = emit_scores_exp(n, *units[ui + 1])
            else:
                nxt_pv = None
            if ui == 0 and pending_final is not None:
                # previous stage's finalize: its deps resolved long ago, so
                # these small PE/DVE ops overlap this stage's ACT-paced units
                emit_finalize(*pending_final)
            if ui == min(1, len(units) - 1):
                # k natural-layout outputs: not needed by any score (those
                # read kT_s directly), so they live here as PE filler
                for j in range(4 * n, 4 * n + 4):
                    emit_k_out(j, kvbuf)
                nc.sync.dma_start(
                    k_d[512 * n : 512 * (n + 1), :].rearrange("(j p) h -> p j h", p=128),
                    kvbuf[:, 0 : 4 * HS].rearrange("p (j h) -> p j h", j=4),
                )
                nc.sync.dma_start(
                    v_d[512 * n : 512 * (n + 1), :].rearrange("(j p) h -> p j h", p=128),
                    kvbuf[:, 4 * HS : 8 * HS].rearrange("p (j h) -> p j h", j=4),
                )
            if ui == ui_rope and nxt_ps is not None:
                # next stage's rope, emitted mid-attention so its latency
                # chain resolves before the stage boundary; only the 8 qk
                # projection fillers must precede it (v fillers spread later)
                fill(16)  # ensure all of next stage's proj is emitted
                qkw_ps = proj_psum.tile(
                    [128, 512], FP32, tag="proj", name=f"qkw_ps{n + 1}"
                )
                kvbuf = kn_pool.tile([128, 8 * HS], BF16, tag="kn", name=f"kvbuf{n + 1}")
                emit_rope(n + 1, nxt_ps[0], nxt_ps[1], qkw_ps, kvbuf)
            rem = len(units) - ui
            rem_f = len(fillers) - fstate["i"]
            fill((rem_f + rem - 1) // rem)
            emit_pv(n, o_ps, *pending_pv)
            pending_pv = nxt_pv
        fill(len(fillers))  # flush any leftovers
        if n + 1 < NT and ui_rope >= len(units):
            qkw_ps = proj_psum.tile([128, 512], FP32, tag="proj", name=f"qkw_ps{n + 1}")
            kvbuf = kn_pool.tile([128, 8 * HS], BF16, tag="kn", name=f"kvbuf{n + 1}")
            emit_rope(n + 1, nxt_ps[0], nxt_ps[1], qkw_ps, kvbuf)
        pending_final = (n, o_ps)
        if nxt_ps is not None:
            qk_ps, v_ps = nxt_ps
    emit_finalize(*pending_final)


_NC_CACHE = {}


def _split_multiwait(nc, max_w=1):
    """Walrus here rejects instructions with >1 semaphore wait. Hoist extra
    waits onto same-engine NoOps inserted immediately before the offender
    (the engine executes its stream in order, so this is semantics-preserving,
    merely stalling slightly earlier)."""
    f = nc.m.functions[0]
    blocks = list(f.blocks)
    tail = blocks[-1].instructions
    for b in blocks:
        insts = b.instructions
        fixed = []
        for inst in insts:
            si = inst.sync_info
            waits = list(si.on_wait) if si and si.on_wait else []
            if len(waits) > max_w:
                for w in waits[:-max_w]:
                    bi = nc.engines[inst.engine].nop()
                    nop = bi.ins
                    for ti in range(len(tail) - 1, -1, -1):
                        if tail[ti] is nop:
                            del tail[ti]
                            break
                    nop.sync_info = mybir.SyncInfo(on_wait=[w], on_update=[])
                    fixed.append(nop)
                si.on_wait = waits[-max_w:]
            fixed.append(inst)
        if len(fixed) != len(insts):
            insts[:] = fixed


def _build_nc():
    if "nc" in _NC_CACHE:
        return _NC_CACHE["nc"]
    from contextlib import ExitStack

    nc = bass.Bass("TRN2", target_bir_lowering=False, debug=False)
    with SplitDrainTileContext(nc) as tc, ExitStack() as ctx:
        _emit(tc, ctx)
    _split_multiwait(nc)
    _NC_CACHE["nc"] = nc
    return nc


def _host_prep(x, Wq, Wk, Wv):
    """Build the per-core input maps (host-side sharding + layout prep)."""
    bf16 = mybir.dt.np(BF16)
    x = np.asarray(x, dtype=np.float32)
    Wq = np.asarray(Wq, dtype=np.float32)
    Wk = np.asarray(Wk, dtype=np.float32)
    Wv = np.asarray(Wv, dtype=np.float32)

    scale = 1.0 / np.sqrt(HS)
    Wc = np.concatenate([Wq * scale, Wk], axis=0)  # [128, C]
    wqkd = np.empty((128, C), dtype=np.float32)  # SBUF image: [k, 128c+m]
    wvd = np.empty((128, NC_CHUNKS * HS), dtype=np.float32)
    for c in range(NC_CHUNKS):
        wqkd[:, 128 * c : 128 * (c + 1)] = Wc[:, 128 * c : 128 * (c + 1)].T
        wvd[:, HS * c : HS * (c + 1)] = Wv[:, 128 * c : 128 * (c + 1)].T

    inv_freq = 1.0 / (10000.0 ** (np.arange(0, HS, 2, dtype=np.float32) / HS))
    t = np.arange(T, dtype=np.float32)
    freqs = np.outer(t, inv_freq)  # [T, 32]
    cos = np.cos(freqs).T.astype(np.float32)  # [32, T]
    sin = np.sin(freqs).T.astype(np.float32)
    t1h = np.repeat(cos, 2, axis=0)  # [64, T], rows 2i and 2i+1 = cos_i
    t2h = np.empty((64, T), dtype=np.float32)
    t2h[0::2] = -sin
    t2h[1::2] = sin
    t1 = t1h.astype(bf16)  # [64, T]; device broadcasts to the k half
    t2 = t2h.astype(bf16)

    permT = np.zeros((128, 128), dtype=np.float32)
    for m in range(128):
        permT[m ^ 1, m] = 1.0

    p = np.arange(128)[:, None]
    cc = np.arange(128)[None, :]
    dmask = (cc >= p).astype(np.float32)

    shared = {
        "wqkd": wqkd.astype(bf16),
        "wvd": wvd.astype(bf16),
        "t1": np.ascontiguousarray(t1),
        "t2": np.ascontiguousarray(t2),
        "permT": permT.astype(bf16),
        "dmask": dmask.astype(bf16),
        "identd": np.eye(128, dtype=np.float32).astype(bf16),
    }
    in_maps = []
    for b in range(NCORES):
        m = dict(shared)
        m["xT"] = np.ascontiguousarray(x[b].T.astype(bf16))  # [C, T]
        in_maps.append(m)
    return in_maps


def run_device(x, Wq, Wk, Wv, trace=False, trace_cores=None):
    """Compile (cached) + run on the 8 NeuronCores. Returns ((out,k,v), raw)."""
    from concourse.bass_utils import run_bass_kernel_spmd

    nc = _build_nc()
    in_maps = _host_prep(x, Wq, Wk, Wv)
    res = run_bass_kernel_spmd(
        nc, in_maps, list(range(NCORES)), trace=trace, trace_cores=trace_cores
    )
    f32 = np.float32
    out = np.stack([res.results[b]["out"].astype(f32) for b in range(NCORES)])
    k = np.stack([res.results[b]["k"].astype(f32) for b in range(NCORES)])
    v = np.stack([res.results[b]["v"].astype(f32) for b in range(NCORES)])
    return (out, k, v), res


def kernel(x, Wq, Wk, Wv):
    (out, k, v), _ = run_device(x, Wq, Wk, Wv, trace=False)
    return out, k, v


def bench_device(x, Wq, Wk, Wv, iters=10):
    """Time the kernel on hardware: repeated executions inside one jit call
    (outputs recycled as the donated output buffers) so the per-call axon
    tunnel RTT amortizes away. Returns (ns_per_iter, (out, k, v))."""
    import time

    import jax
    from jax.sharding import Mesh, PartitionSpec
    from jax.experimental.shard_map import shard_map
    import concourse.bass2jax as bass2jax
    from concourse.bass2jax import _bass_exec_p, install_neuronx_cc_hook

    install_neuronx_cc_hook()
    nc = _build_nc()
    in_maps = _host_prep(x, Wq, Wk, Wv)

    part_name = nc.partition_id_tensor.name if nc.partition_id_tensor else None
    in_names, out_names, out_avals = [], [], []
    for alloc in nc.m.functions[0].allocations:
        if not isinstance(alloc, mybir.MemoryLocationSet):
            continue
        name = alloc.memorylocations[0].name
        if alloc.kind == "ExternalInput":
            if name != part_name:
                in_names.append(name)
        elif alloc.kind == "ExternalOutput":
            out_names.append(name)
            out_avals.append(
                jax.core.ShapedArray(tuple(alloc.tensor_shape), mybir.dt.np(alloc.dtype))
            )
    n_params = len(in_names)
    all_names = in_names + out_names
    if part_name is not None:
        all_names = all_names + [part_name]

    def _one(args, outs):
        ops = list(args) + list(outs)
        if part_name is not None:
            ops.append(bass2jax.partition_id_tensor())
        return _bass_exec_p.bind(
            *ops,
            out_avals=tuple(out_avals),
            in_names=tuple(all_names),
            out_names=tuple(out_names),
            lowering_input_output_aliases=(),
            sim_require_finite=True,
            sim_require_nnan=True,
            nc=nc,
        )

    def _body(*ops):
        args, outs = ops[:n_params], list(ops[n_params:])
        return tuple(_one(args, outs))

    devices = jax.devices()[:NCORES]
    mesh = Mesh(np.asarray(devices), ("core",))
    nin = n_params + len(out_names)
    sharded = jax.jit(
        shard_map(
            _body,
            mesh=mesh,
            in_specs=(PartitionSpec("core"),) * nin,
            out_specs=(PartitionSpec("core"),) * len(out_names),
            check_rep=False,
        ),
        donate_argnums=tuple(range(n_params, nin)),
        keep_unused=True,
    )
    concat_in = [
        np.concatenate([np.asarray(in_maps[c][nm]) for c in range(NCORES)], axis=0)
        for nm in in_names
    ]
    concat_zeros = [
        np.zeros((NCORES * av.shape[0], *av.shape[1:]), av.dtype) for av in out_avals
    ]
    concat_in = [jax.device_put(a) for a in concat_in]
    outs = sharded(*concat_in, *concat_zeros)  # compile + warmup
    jax.block_until_ready(outs)
    first = [np.asarray(o) for o in outs]
    best = float("inf")
    for _ in range(iters):
        t0 = time.perf_counter()
        outs = sharded(*concat_in, *outs)
        jax.block_until_ready(outs)
        best = min(best, time.perf_counter() - t0)
    res = [
        first[i].reshape(NCORES, *out_avals[i].shape).astype(np.float32)
        for i in range(len(out_names))
    ]
    by = dict(zip(out_names, res))
    return best * 1e9, (by["out"], by["k"], by["v"])
